# revision 1
# baseline (speedup 1.0000x reference)
"""GNN message-passing (CompGCN-style edge-softmax) Trainium2 kernel.

Contract: kernel(**inputs) takes FULL unsharded inputs (ent_emb [50000,128] f32,
rel_emb [1000,128] f32, neigh_w [128,128] f32, src/dst/rel_id [600000] int) and
returns the FULL [50000,128] f32 output of:

    comp  = ent_emb[src] * rel_emb[rel_id]
    score = sum(comp * ent_emb[dst], -1)
    alpha = segment_softmax(score, dst)          # grouped by dst
    neigh = segment_sum(comp * alpha[:,None], dst)
    out   = tanh(neigh @ neigh_w)

Sharding: edges are sharded across the 8 cores BY DST RANGE (core c owns dst in
[c*6250,(c+1)*6250)), so segment max/sum are purely core-local and no
collective is needed; ent_emb/rel_emb/neigh_w are replicated.  Within a core,
edges are grouped by 128-node dst blocks; per block the kernel bulk-gathers
ent_emb[src] / rel_emb[rel] / ent_emb[dst] rows with dma_gather, computes
score -> es=exp(score) (segment-max subtraction is skipped: |score| <~ 60 <<
88 so exp cannot overflow, and alpha = es/sum(es) is exact), builds the
weighted one-hot W[e,j] = es_e * (dst_e == j) on DVE, and accumulates
    accT[h,j] += comp_c.T @ W_c      (TensorE, PSUM)
    den[j]    += W_c.T @ ones        (TensorE, PSUM)
then out_block = tanh((accT.T @ neigh_w) / den) and a contiguous DMA out.

dma_gather indices are int16, so ent_emb (50000 rows) src-gathers are split
into a lo (rows < 32768) and hi (rows >= 32768) gather per block, with edges
sorted by src inside each section (ascending HBM addresses).  Padded slots
repeat a real edge but carry dst_oh = 128 which matches no iota column, so
they contribute exactly zero to den/acc.
"""

import numpy as np

N_ENT = 50000
N_REL = 1000
N_EDGES = 600000
H = 128
P = 128
N_CORES = 8
NPC = N_ENT // N_CORES          # nodes per core
LO_ROWS = 32768                 # int16 gather split point

_cache = {}


def _build_program(npc, n_blocks, s_lo, s_hi, n_ent, n_rel, lo_rows):
    """Build the SPMD Bass/Tile program for one core shape."""
    import concourse.bacc as bacc
    import concourse.mybir as mybir
    import concourse.tile as tile

    f32 = mybir.dt.float32
    i16 = mybir.dt.int16
    S = s_lo + s_hi

    nc = bacc.Bacc("TRN2", target_bir_lowering=False, debug=False,
                   num_devices=N_CORES)

    ent = nc.dram_tensor("ent", [n_ent, H], f32, kind="ExternalInput")
    # this core's own node slice (dst rows) — per-core data, static local base
    ent_loc = nc.dram_tensor("ent_loc", [npc, H], f32, kind="ExternalInput")
    relt = nc.dram_tensor("relt", [n_rel, H], f32, kind="ExternalInput")
    w_in = nc.dram_tensor("w", [H, H], f32, kind="ExternalInput")
    iota_in = nc.dram_tensor("iota", [P, P], f32, kind="ExternalInput")
    sgi_in = nc.dram_tensor("src_gi", [P, n_blocks, S * 8], i16,
                            kind="ExternalInput")
    rgi_in = nc.dram_tensor("rel_gi", [P, n_blocks, S * 8], i16,
                            kind="ExternalInput")
    dgi_in = nc.dram_tensor("dst_gi", [P, n_blocks, S * 8], i16,
                            kind="ExternalInput")
    doh_in = nc.dram_tensor("dst_oh", [P, n_blocks, S], f32,
                            kind="ExternalInput")
    out = nc.dram_tensor("out", [npc, H], f32, kind="ExternalOutput")

    import concourse.bass as bass

    def bc(ap, dims):
        # append/insert stride-0 dims: dims is the final [step,count] list
        return bass.AP(ap.tensor, ap.offset, dims)

    with tile.TileContext(nc) as tc:
        with (
            tc.tile_pool(name="const", bufs=1) as constp,
            tc.tile_pool(name="idx", bufs=1) as idxp,
            tc.tile_pool(name="data", bufs=2) as datap,
            tc.tile_pool(name="small", bufs=2) as smallp,
            tc.tile_pool(name="psum", bufs=2, space="PSUM") as psump,
            tc.tile_pool(name="psum1", bufs=2, space="PSUM") as psum1p,
        ):
            iota_t = constp.tile([P, P], f32)
            nc.sync.dma_start(iota_t[:], iota_in[:])
            w_t = constp.tile([H, H], f32)
            nc.sync.dma_start(w_t[:], w_in[:])
            ones_t = constp.tile([P, 1], f32)
            nc.vector.memset(ones_t[:], 1.0)

            sgi_t = idxp.tile([P, n_blocks, S * 8], i16)
            nc.sync.dma_start(sgi_t[:], sgi_in[:])
            rgi_t = idxp.tile([P, n_blocks, S * 8], i16)
            nc.sync.dma_start(rgi_t[:], rgi_in[:])
            dgi_t = idxp.tile([P, n_blocks, S * 8], i16)
            nc.sync.dma_start(dgi_t[:], dgi_in[:])
            doh_t = idxp.tile([P, n_blocks, S], f32)
            nc.sync.dma_start(doh_t[:], doh_in[:])

            for b in range(n_blocks):
                base = b * P
                nodes_b = min(P, npc - base)

                src_rows = datap.tile([P, S, H], f32, tag="src")
                rel_rows = datap.tile([P, S, H], f32, tag="rel")
                dst_rows = datap.tile([P, S, H], f32, tag="dst")
                w_oh = datap.tile([P, S, H], f32, tag="W")

                if s_lo > 0:
                    nc.gpsimd.dma_gather(
                        src_rows[:, 0:s_lo, :], ent[0:lo_rows, :],
                        sgi_t[:, b, 0:s_lo * 8], s_lo * P, s_lo * P, H,
                        single_packet=False)
                if s_hi > 0:
                    nc.gpsimd.dma_gather(
                        src_rows[:, s_lo:S, :], ent[lo_rows:n_ent, :],
                        sgi_t[:, b, s_lo * 8:S * 8], s_hi * P, s_hi * P, H,
                        single_packet=False)
                nc.gpsimd.dma_gather(
                    rel_rows[:, :, :], relt[:, :],
                    rgi_t[:, b, :], S * P, S * P, H, single_packet=False)
                nc.gpsimd.dma_gather(
                    dst_rows[:, :, :], ent_loc[base:base + nodes_b, :],
                    dgi_t[:, b, :], S * P, S * P, H, single_packet=False)

                # comp = ent[src] * rel[rel_id]   (in-place over src_rows)
                nc.vector.tensor_tensor(
                    out=src_rows[:], in0=src_rows[:], in1=rel_rows[:],
                    op=mybir.AluOpType.mult)
                # prod = comp * ent[dst]          (in-place over dst_rows)
                nc.vector.tensor_tensor(
                    out=dst_rows[:], in0=src_rows[:], in1=dst_rows[:],
                    op=mybir.AluOpType.mult)
                score = smallp.tile([P, S], f32, tag="score")
                nc.vector.tensor_reduce(
                    out=score[:], in_=dst_rows[:],
                    axis=mybir.AxisListType.X, op=mybir.AluOpType.add)
                es = smallp.tile([P, S], f32, tag="es")
                nc.scalar.activation(
                    out=es[:], in_=score[:],
                    func=mybir.ActivationFunctionType.Exp)

                # one-hot: W[p, c, j] = (dst_oh[p, c] == j)
                doh_ap = doh_t[:, b, :]
                doh_b = bc(doh_ap, [doh_ap.ap[0], doh_ap.ap[1], [0, H]])
                iota_ap = iota_t[:]
                iota_b = bc(iota_ap, [iota_ap.ap[0], [0, S], iota_ap.ap[1]])
                nc.vector.tensor_tensor(
                    out=w_oh[:], in0=doh_b, in1=iota_b,
                    op=mybir.AluOpType.is_equal)
                # W *= es  (broadcast es over the one-hot columns)
                es_ap = es[:]
                es_b = bc(es_ap, [es_ap.ap[0], es_ap.ap[1], [0, H]])
                nc.vector.tensor_tensor(
                    out=w_oh[:], in0=w_oh[:], in1=es_b,
                    op=mybir.AluOpType.mult)

                # accT[h, j] = sum_c comp_c.T @ W_c
                acct_ps = psump.tile([P, P], f32, tag="accT")
                for c in range(S):
                    nc.tensor.matmul(
                        acct_ps[:], lhsT=src_rows[:, c, :], rhs=w_oh[:, c, :],
                        start=(c == 0), stop=(c == S - 1))
                # den[j] = sum_c W_c.T @ ones
                den_ps = psum1p.tile([P, 1], f32, tag="den")
                for c in range(S):
                    nc.tensor.matmul(
                        den_ps[:], lhsT=w_oh[:, c, :], rhs=ones_t[:],
                        start=(c == 0), stop=(c == S - 1))

                acct_sb = smallp.tile([P, P], f32, tag="acct_sb")
                nc.scalar.copy(acct_sb[:], acct_ps[:])
                den_sb = smallp.tile([P, 1], f32, tag="den_sb")
                nc.vector.tensor_scalar_max(den_sb[:], den_ps[:], 1e-30)
                rden = smallp.tile([P, 1], f32, tag="rden")
                nc.vector.reciprocal(rden[:], den_sb[:])

                out_ps = psump.tile([P, H], f32, tag="out_ps")
                nc.tensor.matmul(out_ps[:], lhsT=acct_sb[:], rhs=w_t[:],
                                 start=True, stop=True)
                out_sb = smallp.tile([P, H], f32, tag="out_sb")
                nc.scalar.activation(
                    out=out_sb[:], in_=out_ps[:],
                    func=mybir.ActivationFunctionType.Tanh, scale=rden[:])
                nc.sync.dma_start(out[base:base + nodes_b, :],
                                  out_sb[:nodes_b, :])

    nc.compile()
    return nc


def _idx_to_gather_layout(arr):
    """[S*128] int16 gather-position-ordered indices -> [128, S*8] tile."""
    a = arr.reshape(-1, 16).T.astype(np.int16)      # [16, S*8]
    return np.tile(a, (8, 1))                        # [128, S*8]


def _prep_inputs(ent_emb, rel_emb, neigh_w, src, dst, rel_id):
    """Partition edges by dst core/block, build per-core gather index arrays.

    Returns (in_maps, shape_key) where shape_key parameterizes the program.
    """
    src = np.asarray(src).astype(np.int64)
    dst = np.asarray(dst).astype(np.int64)
    rel_id = np.asarray(rel_id).astype(np.int64)
    n_blocks = (NPC + P - 1) // P

    order = np.argsort(dst, kind="stable")
    src_s, dst_s, rel_s = src[order], dst[order], rel_id[order]
    # per-(core,block) group id; monotone in dst since blocks nest in cores
    g_s = (dst_s // NPC) * n_blocks + (dst_s % NPC) // P
    n_gblocks = N_CORES * n_blocks
    bounds = np.searchsorted(g_s, np.arange(n_gblocks + 1))

    # first pass: per-block lo/hi counts -> global S_LO / S_HI
    max_lo = 1
    max_hi = 1
    lohi = []
    for g in range(n_gblocks):
        e0, e1 = bounds[g], bounds[g + 1]
        s_g = src_s[e0:e1]
        n_lo = int((s_g < LO_ROWS).sum())
        n_hi = int(e1 - e0 - n_lo)
        lohi.append((e0, e1, n_lo, n_hi))
        max_lo = max(max_lo, n_lo)
        max_hi = max(max_hi, n_hi)
    s_lo = (max_lo + P - 1) // P
    s_hi = (max_hi + P - 1) // P
    S = s_lo + s_hi

    in_maps = []
    for c in range(N_CORES):
        sgi = np.zeros((n_blocks, S * P), np.int16)
        rgi = np.zeros((n_blocks, S * P), np.int16)
        dgi = np.zeros((n_blocks, S * P), np.int16)
        doh = np.full((n_blocks, S * P), float(P), np.float32)
        for b in range(n_blocks):
            g = c * n_blocks + b
            e0, e1, n_lo, n_hi = lohi[g]
            base = c * NPC + b * P
            s_g, d_g, r_g = src_s[e0:e1], dst_s[e0:e1], rel_s[e0:e1]
            is_lo = s_g < LO_ROWS
            for sel, off, cap, sub in ((is_lo, 0, s_lo * P, 0),
                                       (~is_lo, s_lo * P, s_hi * P, LO_ROWS)):
                ss, dd, rr = s_g[sel], d_g[sel], r_g[sel]
                o2 = np.argsort(ss, kind="stable")
                ss, dd, rr = ss[o2], dd[o2], rr[o2]
                n = len(ss)
                assert n <= cap
                sgi[b, off:off + n] = ss - sub
                rgi[b, off:off + n] = rr
                dgi[b, off:off + n] = dd - base
                doh[b, off:off + n] = (dd - base).astype(np.float32)
                if n < cap:  # pad with a repeat of a real edge (or zeros)
                    if n > 0:
                        sgi[b, off + n:off + cap] = ss[0] - sub
                        rgi[b, off + n:off + cap] = rr[0]
                        dgi[b, off + n:off + cap] = dd[0] - base
                    # doh stays 128 -> zero contribution
        # to device layouts
        sgi_l = np.stack([_idx_to_gather_layout(sgi[b]) for b in range(n_blocks)])
        rgi_l = np.stack([_idx_to_gather_layout(rgi[b]) for b in range(n_blocks)])
        dgi_l = np.stack([_idx_to_gather_layout(dgi[b]) for b in range(n_blocks)])
        doh_l = np.stack([doh[b].reshape(S, P).T for b in range(n_blocks)])
        iota = np.broadcast_to(np.arange(P, dtype=np.float32), (P, P)).copy()
        in_maps.append({
            "ent": np.ascontiguousarray(ent_emb, np.float32),
            "ent_loc": np.ascontiguousarray(
                ent_emb[c * NPC:(c + 1) * NPC], np.float32),
            "relt": np.ascontiguousarray(rel_emb, np.float32),
            "w": np.ascontiguousarray(neigh_w, np.float32),
            "iota": iota,
            "src_gi": np.ascontiguousarray(sgi_l.transpose(1, 0, 2)),
            "rel_gi": np.ascontiguousarray(rgi_l.transpose(1, 0, 2)),
            "dst_gi": np.ascontiguousarray(dgi_l.transpose(1, 0, 2)),
            "dst_oh": np.ascontiguousarray(
                doh_l.transpose(1, 0, 2).astype(np.float32)),
        })
    return in_maps, (NPC, n_blocks, s_lo, s_hi, N_ENT, N_REL, LO_ROWS)


LAST_RESULT = None


def _install_ntff_hook():
    """Provide the antenv.axon_hooks module the container's stub lacks, so
    run_bass_kernel_spmd(trace=True) can capture NTFF profiles via libaxon."""
    import sys
    import types
    if "antenv.axon_hooks" in sys.modules:
        return
    mod = types.ModuleType("antenv.axon_hooks")
    hook = [None]
    mod.set_axon_ntff_profile_hook = lambda h: hook.__setitem__(0, h)
    mod.get_axon_ntff_profile_hook = lambda: hook[0]
    sys.modules["antenv.axon_hooks"] = mod
    import antenv
    antenv.axon_hooks = mod
    try:
        from trn_agent_boot.trn_boot import _ntff_profile_via_ctypes
        h = _ntff_profile_via_ctypes("/opt/axon/libaxon_pjrt.so")
        if h is not None:
            mod.set_axon_ntff_profile_hook(lambda *a, **k: h(*a, **k))
    except Exception as e:  # degrade to no-trace
        print("ntff hook install failed:", e)


def kernel(ent_emb, rel_emb, neigh_w, src, dst, rel_id, _trace=False):
    global LAST_RESULT
    from concourse.bass_utils import run_bass_kernel_spmd
    if _trace:
        _install_ntff_hook()

    in_maps, key = _prep_inputs(ent_emb, rel_emb, neigh_w, src, dst, rel_id)
    if key not in _cache:
        _cache[key] = _build_program(*key)
    nc = _cache[key]
    res = run_bass_kernel_spmd(nc, in_maps, list(range(N_CORES)),
                               trace=_trace)
    LAST_RESULT = res
    return np.concatenate([r["out"] for r in res.results], axis=0)



# revision 6
# speedup vs baseline: 1.3744x; 1.3744x over previous
"""GNN message-passing (CompGCN-style edge-softmax) Trainium2 kernel.

Contract: kernel(**inputs) takes FULL unsharded inputs (ent_emb [50000,128] f32,
rel_emb [1000,128] f32, neigh_w [128,128] f32, src/dst/rel_id [600000] int) and
returns the FULL [50000,128] f32 output of:

    comp  = ent_emb[src] * rel_emb[rel_id]
    score = sum(comp * ent_emb[dst], -1)
    alpha = segment_softmax(score, dst)          # grouped by dst
    neigh = segment_sum(comp * alpha[:,None], dst)
    out   = tanh(neigh @ neigh_w)

Sharding: edges are sharded across the 8 cores BY DST RANGE (core c owns dst in
[c*6250,(c+1)*6250)), so segment max/sum are purely core-local and no
collective is needed; ent_emb/rel_emb/neigh_w are replicated.  Within a core,
edges are grouped by 128-node dst blocks; per block the kernel bulk-gathers
ent_emb[src] / rel_emb[rel] / ent_emb[dst] rows with dma_gather, computes
score -> es=exp(score) (segment-max subtraction is skipped: |score| <~ 60 <<
88 so exp cannot overflow, and alpha = es/sum(es) is exact), builds the
weighted one-hot W[e,j] = es_e * (dst_e == j) on DVE, and accumulates
    accT[h,j] += comp_c.T @ W_c      (TensorE, PSUM)
    den[j]    += W_c.T @ ones        (TensorE, PSUM)
then out_block = tanh((accT.T @ neigh_w) / den) and a contiguous DMA out.

dma_gather indices are int16, so ent_emb (50000 rows) src-gathers are split
into a lo (rows < 32768) and hi (rows >= 32768) gather per block, with edges
sorted by src inside each section (ascending HBM addresses).  Padded slots
repeat a real edge but carry dst_oh = 128 which matches no iota column, so
they contribute exactly zero to den/acc.
"""

import numpy as np

N_ENT = 50000
N_REL = 1000
N_EDGES = 600000
H = 128
P = 128
N_CORES = 8
NPC = N_ENT // N_CORES          # nodes per core
LO_ROWS = 32768                 # int16 gather split point

_cache = {}


def _build_program(npc, n_blocks, s_lo, s_hi, n_ent, n_rel, lo_rows):
    """Build the SPMD Bass/Tile program for one core shape."""
    import concourse.bacc as bacc
    import concourse.mybir as mybir
    import concourse.tile as tile

    f32 = mybir.dt.float32
    i16 = mybir.dt.int16
    S = s_lo + s_hi

    nc = bacc.Bacc("TRN2", target_bir_lowering=False, debug=False,
                   num_devices=N_CORES)

    ent = nc.dram_tensor("ent", [n_ent, H], f32, kind="ExternalInput")
    # this core's own node slice (dst rows) — per-core data, static local base
    ent_loc = nc.dram_tensor("ent_loc", [npc, H], f32, kind="ExternalInput")
    relt = nc.dram_tensor("relt", [n_rel, H], f32, kind="ExternalInput")
    w_in = nc.dram_tensor("w", [H, H], f32, kind="ExternalInput")
    iota_in = nc.dram_tensor("iota", [P, P], f32, kind="ExternalInput")
    sgi_in = nc.dram_tensor("src_gi", [P, n_blocks, S * 8], i16,
                            kind="ExternalInput")
    rgi_in = nc.dram_tensor("rel_gi", [P, n_blocks, S * 8], i16,
                            kind="ExternalInput")
    doh_in = nc.dram_tensor("dst_oh", [P, n_blocks, S], f32,
                            kind="ExternalInput")
    bnd_in = nc.dram_tensor("bounds", [P, n_blocks, 4], f32,
                            kind="ExternalInput")
    iote_in = nc.dram_tensor("iota_e", [P, S * P], f32,
                             kind="ExternalInput")
    out = nc.dram_tensor("out", [npc, H], f32, kind="ExternalOutput")

    import concourse.bass as bass

    def bc(ap, dims):
        # append/insert stride-0 dims: dims is the final [step,count] list
        return bass.AP(ap.tensor, ap.offset, dims)

    with tile.TileContext(nc) as tc:
        with (
            tc.tile_pool(name="const", bufs=1) as constp,
            tc.tile_pool(name="idx", bufs=1) as idxp,
            tc.tile_pool(name="data", bufs=2) as datap,
            tc.tile_pool(name="small", bufs=2) as smallp,
            tc.tile_pool(name="psum", bufs=2, space="PSUM") as psump,
            tc.tile_pool(name="psum1", bufs=2, space="PSUM") as psum1p,
            tc.tile_pool(name="psum2", bufs=2, space="PSUM") as psum2p,
        ):
            iota_t = constp.tile([P, P], f32)
            nc.sync.dma_start(iota_t[:], iota_in[:])
            w_t = constp.tile([H, H], f32)
            nc.sync.dma_start(w_t[:], w_in[:])
            ones_t = constp.tile([P, 1], f32)
            nc.vector.memset(ones_t[:], 1.0)

            sgi_t = idxp.tile([P, n_blocks, S * 8], i16)
            nc.sync.dma_start(sgi_t[:], sgi_in[:])
            rgi_t = idxp.tile([P, n_blocks, S * 8], i16)
            nc.sync.dma_start(rgi_t[:], rgi_in[:])
            doh_t = idxp.tile([P, n_blocks, S], f32)
            nc.sync.dma_start(doh_t[:], doh_in[:])
            bnd_t = idxp.tile([P, n_blocks, 4], f32)
            nc.sync.dma_start(bnd_t[:], bnd_in[:])
            iote_t = constp.tile([P, S * P], f32)
            nc.sync.dma_start(iote_t[:], iote_in[:])

            for b in range(n_blocks):
                base = b * P
                nodes_b = min(P, npc - base)

                src_rows = datap.tile([P, S, H], f32, tag="src")
                rel_rows = datap.tile([P, S, H], f32, tag="rel")
                dst_rows = datap.tile([P, S, H], f32, tag="dst")
                w_oh = datap.tile([P, S, H], f32, tag="W")

                if s_lo > 0:
                    nc.gpsimd.dma_gather(
                        src_rows[:, 0:s_lo, :], ent[0:lo_rows, :],
                        sgi_t[:, b, 0:s_lo * 8], s_lo * P, s_lo * P, H,
                        single_packet=False)
                if s_hi > 0:
                    nc.gpsimd.dma_gather(
                        src_rows[:, s_lo:S, :], ent[lo_rows:n_ent, :],
                        sgi_t[:, b, s_lo * 8:S * 8], s_hi * P, s_hi * P, H,
                        single_packet=False)
                nc.gpsimd.dma_gather(
                    rel_rows[:, :, :], relt[:, :],
                    rgi_t[:, b, :], S * P, S * P, H, single_packet=False)

                # dst rows: contiguous 128-row block load + one-hot PE expand.
                # OHT[j, e] = 1 iff slot e's dst == j; each node's slots are
                # two contiguous runs (lo/hi section), given by host bounds.
                eblk = datap.tile([P, H], f32, tag="eblk")
                if b < 2 or nodes_b < P:
                    nc.vector.memset(eblk[:], 0.0)
                nc.sync.dma_start(eblk[:nodes_b, :],
                                  ent_loc[base:base + nodes_b, :])
                oht = datap.tile([P, S * P], f32, tag="oht")
                t_a = datap.tile([P, S * P], f32, tag="t_a")
                t_b = datap.tile([P, S * P], f32, tag="t_b")

                def bnd(k):
                    ap = bnd_t[:, b, k:k + 1]
                    return bc(ap, [ap.ap[0], [0, S * P]])
                nc.vector.tensor_tensor(out=t_a[:], in0=iote_t[:],
                                        in1=bnd(0), op=mybir.AluOpType.is_ge)
                nc.vector.tensor_tensor(out=oht[:], in0=iote_t[:],
                                        in1=bnd(1), op=mybir.AluOpType.is_lt)
                nc.vector.tensor_tensor(out=t_a[:], in0=t_a[:], in1=oht[:],
                                        op=mybir.AluOpType.mult)
                nc.vector.tensor_tensor(out=t_b[:], in0=iote_t[:],
                                        in1=bnd(2), op=mybir.AluOpType.is_ge)
                nc.vector.tensor_tensor(out=oht[:], in0=iote_t[:],
                                        in1=bnd(3), op=mybir.AluOpType.is_lt)
                nc.vector.tensor_tensor(out=t_b[:], in0=t_b[:], in1=oht[:],
                                        op=mybir.AluOpType.mult)
                nc.vector.tensor_tensor(out=oht[:], in0=t_a[:], in1=t_b[:],
                                        op=mybir.AluOpType.add)
                for c in range(S):
                    dst_ps = psum2p.tile([P, H], f32, tag="dst_ps")
                    nc.tensor.matmul(
                        dst_ps[:], lhsT=oht[:, c * P:(c + 1) * P],
                        rhs=eblk[:], start=True, stop=True)
                    nc.scalar.copy(dst_rows[:, c, :], dst_ps[:])

                # comp = ent[src] * rel[rel_id]   (in-place over src_rows)
                nc.vector.tensor_tensor(
                    out=src_rows[:], in0=src_rows[:], in1=rel_rows[:],
                    op=mybir.AluOpType.mult)
                # prod = comp * ent[dst]          (in-place over dst_rows)
                nc.vector.tensor_tensor(
                    out=dst_rows[:], in0=src_rows[:], in1=dst_rows[:],
                    op=mybir.AluOpType.mult)
                score = smallp.tile([P, S], f32, tag="score")
                nc.vector.tensor_reduce(
                    out=score[:], in_=dst_rows[:],
                    axis=mybir.AxisListType.X, op=mybir.AluOpType.add)
                es = smallp.tile([P, S], f32, tag="es")
                nc.scalar.activation(
                    out=es[:], in_=score[:],
                    func=mybir.ActivationFunctionType.Exp)

                # one-hot: W[p, c, j] = (dst_oh[p, c] == j)
                doh_ap = doh_t[:, b, :]
                doh_b = bc(doh_ap, [doh_ap.ap[0], doh_ap.ap[1], [0, H]])
                iota_ap = iota_t[:]
                iota_b = bc(iota_ap, [iota_ap.ap[0], [0, S], iota_ap.ap[1]])
                nc.vector.tensor_tensor(
                    out=w_oh[:], in0=doh_b, in1=iota_b,
                    op=mybir.AluOpType.is_equal)
                # W *= es  (broadcast es over the one-hot columns)
                es_ap = es[:]
                es_b = bc(es_ap, [es_ap.ap[0], es_ap.ap[1], [0, H]])
                nc.vector.tensor_tensor(
                    out=w_oh[:], in0=w_oh[:], in1=es_b,
                    op=mybir.AluOpType.mult)

                # accT[h, j] = sum_c comp_c.T @ W_c
                acct_ps = psump.tile([P, P], f32, tag="accT")
                for c in range(S):
                    nc.tensor.matmul(
                        acct_ps[:], lhsT=src_rows[:, c, :], rhs=w_oh[:, c, :],
                        start=(c == 0), stop=(c == S - 1))
                # den[j] = sum_c W_c.T @ ones
                den_ps = psum1p.tile([P, 1], f32, tag="den")
                for c in range(S):
                    nc.tensor.matmul(
                        den_ps[:], lhsT=w_oh[:, c, :], rhs=ones_t[:],
                        start=(c == 0), stop=(c == S - 1))

                acct_sb = smallp.tile([P, P], f32, tag="acct_sb")
                nc.scalar.copy(acct_sb[:], acct_ps[:])
                den_sb = smallp.tile([P, 1], f32, tag="den_sb")
                nc.vector.tensor_scalar_max(den_sb[:], den_ps[:], 1e-30)
                rden = smallp.tile([P, 1], f32, tag="rden")
                nc.vector.reciprocal(rden[:], den_sb[:])

                out_ps = psump.tile([P, H], f32, tag="out_ps")
                nc.tensor.matmul(out_ps[:], lhsT=acct_sb[:], rhs=w_t[:],
                                 start=True, stop=True)
                out_sb = smallp.tile([P, H], f32, tag="out_sb")
                nc.scalar.activation(
                    out=out_sb[:], in_=out_ps[:],
                    func=mybir.ActivationFunctionType.Tanh, scale=rden[:])
                nc.sync.dma_start(out[base:base + nodes_b, :],
                                  out_sb[:nodes_b, :])

    nc.compile()
    return nc


def _idx_to_gather_layout(arr):
    """[S*128] int16 gather-position-ordered indices -> [128, S*8] tile."""
    a = arr.reshape(-1, 16).T.astype(np.int16)      # [16, S*8]
    return np.tile(a, (8, 1))                        # [128, S*8]


def _prep_inputs(ent_emb, rel_emb, neigh_w, src, dst, rel_id):
    """Partition edges by dst core/block, build per-core gather index arrays.

    Returns (in_maps, shape_key) where shape_key parameterizes the program.
    """
    src = np.asarray(src).astype(np.int64)
    dst = np.asarray(dst).astype(np.int64)
    rel_id = np.asarray(rel_id).astype(np.int64)
    n_blocks = (NPC + P - 1) // P

    order = np.argsort(dst, kind="stable")
    src_s, dst_s, rel_s = src[order], dst[order], rel_id[order]
    # per-(core,block) group id; monotone in dst since blocks nest in cores
    g_s = (dst_s // NPC) * n_blocks + (dst_s % NPC) // P
    n_gblocks = N_CORES * n_blocks
    bounds = np.searchsorted(g_s, np.arange(n_gblocks + 1))

    # first pass: per-block lo/hi counts -> global S_LO / S_HI
    max_lo = 1
    max_hi = 1
    lohi = []
    for g in range(n_gblocks):
        e0, e1 = bounds[g], bounds[g + 1]
        s_g = src_s[e0:e1]
        n_lo = int((s_g < LO_ROWS).sum())
        n_hi = int(e1 - e0 - n_lo)
        lohi.append((e0, e1, n_lo, n_hi))
        max_lo = max(max_lo, n_lo)
        max_hi = max(max_hi, n_hi)
    s_lo = (max_lo + P - 1) // P
    s_hi = (max_hi + P - 1) // P
    S = s_lo + s_hi

    in_maps = []
    for c in range(N_CORES):
        sgi = np.zeros((n_blocks, S * P), np.int16)
        rgi = np.zeros((n_blocks, S * P), np.int16)
        doh = np.full((n_blocks, S * P), float(P), np.float32)
        bnd = np.zeros((n_blocks, P, 4), np.float32)
        for b in range(n_blocks):
            g = c * n_blocks + b
            e0, e1, n_lo, n_hi = lohi[g]
            base = c * NPC + b * P
            s_g, d_g, r_g = src_s[e0:e1], dst_s[e0:e1], rel_s[e0:e1]
            is_lo = s_g < LO_ROWS
            for si, (sel, off, cap, sub) in enumerate(
                    ((is_lo, 0, s_lo * P, 0),
                     (~is_lo, s_lo * P, s_hi * P, LO_ROWS))):
                ss, dd, rr = s_g[sel], d_g[sel], r_g[sel]
                o2 = np.argsort(dd, kind="stable")  # dst-sorted: runs per node
                ss, dd, rr = ss[o2], dd[o2], rr[o2]
                n = len(ss)
                assert n <= cap
                sgi[b, off:off + n] = ss - sub
                rgi[b, off:off + n] = rr
                doh[b, off:off + n] = (dd - base).astype(np.float32)
                cum = np.searchsorted(dd - base, np.arange(P + 1)) + off
                bnd[b, :, 2 * si] = cum[:-1]
                bnd[b, :, 2 * si + 1] = cum[1:]
                if n < cap:  # pad with a repeat of a real edge (or zeros)
                    if n > 0:
                        sgi[b, off + n:off + cap] = ss[0] - sub
                        rgi[b, off + n:off + cap] = rr[0]
                    # doh stays 128 -> zero contribution
        # to device layouts
        sgi_l = np.stack([_idx_to_gather_layout(sgi[b]) for b in range(n_blocks)])
        rgi_l = np.stack([_idx_to_gather_layout(rgi[b]) for b in range(n_blocks)])
        doh_l = np.stack([doh[b].reshape(S, P).T for b in range(n_blocks)])
        iota = np.broadcast_to(np.arange(P, dtype=np.float32), (P, P)).copy()
        iota_e = np.broadcast_to(
            np.arange(S * P, dtype=np.float32), (P, S * P)).copy()
        in_maps.append({
            "ent": np.ascontiguousarray(ent_emb, np.float32),
            "ent_loc": np.ascontiguousarray(
                ent_emb[c * NPC:(c + 1) * NPC], np.float32),
            "relt": np.ascontiguousarray(rel_emb, np.float32),
            "w": np.ascontiguousarray(neigh_w, np.float32),
            "iota": iota,
            "iota_e": iota_e,
            "src_gi": np.ascontiguousarray(sgi_l.transpose(1, 0, 2)),
            "rel_gi": np.ascontiguousarray(rgi_l.transpose(1, 0, 2)),
            "bounds": np.ascontiguousarray(bnd.transpose(1, 0, 2)),
            "dst_oh": np.ascontiguousarray(
                doh_l.transpose(1, 0, 2).astype(np.float32)),
        })
    return in_maps, (NPC, n_blocks, s_lo, s_hi, N_ENT, N_REL, LO_ROWS)


LAST_RESULT = None


def _install_ntff_hook():
    """Provide the antenv.axon_hooks module the container's stub lacks, so
    run_bass_kernel_spmd(trace=True) can capture NTFF profiles via libaxon."""
    import sys
    import types
    if "antenv.axon_hooks" in sys.modules:
        return
    mod = types.ModuleType("antenv.axon_hooks")
    hook = [None]
    mod.set_axon_ntff_profile_hook = lambda h: hook.__setitem__(0, h)
    mod.get_axon_ntff_profile_hook = lambda: hook[0]
    sys.modules["antenv.axon_hooks"] = mod
    import antenv
    antenv.axon_hooks = mod
    try:
        from trn_agent_boot.trn_boot import _ntff_profile_via_ctypes
        h = _ntff_profile_via_ctypes("/opt/axon/libaxon_pjrt.so")
        if h is not None:
            mod.set_axon_ntff_profile_hook(lambda *a, **k: h(*a, **k))
    except Exception as e:  # degrade to no-trace
        print("ntff hook install failed:", e)


def kernel(ent_emb, rel_emb, neigh_w, src, dst, rel_id, _trace=False):
    global LAST_RESULT
    from concourse.bass_utils import run_bass_kernel_spmd
    if _trace:
        _install_ntff_hook()

    in_maps, key = _prep_inputs(ent_emb, rel_emb, neigh_w, src, dst, rel_id)
    if key not in _cache:
        _cache[key] = _build_program(*key)
    nc = _cache[key]
    res = run_bass_kernel_spmd(nc, in_maps, list(range(N_CORES)),
                               trace=_trace)
    LAST_RESULT = res
    return np.concatenate([r["out"] for r in res.results], axis=0)



# revision 9
# speedup vs baseline: 1.3818x; 1.0054x over previous
"""GNN message-passing (CompGCN-style edge-softmax) Trainium2 kernel.

Contract: kernel(**inputs) takes FULL unsharded inputs (ent_emb [50000,128] f32,
rel_emb [1000,128] f32, neigh_w [128,128] f32, src/dst/rel_id [600000] int) and
returns the FULL [50000,128] f32 output of:

    comp  = ent_emb[src] * rel_emb[rel_id]
    score = sum(comp * ent_emb[dst], -1)
    alpha = segment_softmax(score, dst)          # grouped by dst
    neigh = segment_sum(comp * alpha[:,None], dst)
    out   = tanh(neigh @ neigh_w)

Sharding: edges are sharded across the 8 cores BY DST RANGE (core c owns dst in
[c*6250,(c+1)*6250)), so segment max/sum are purely core-local and no
collective is needed; ent_emb/rel_emb/neigh_w are replicated.  Within a core,
edges are grouped by 128-node dst blocks; per block the kernel bulk-gathers
ent_emb[src] / rel_emb[rel] / ent_emb[dst] rows with dma_gather, computes
score -> es=exp(score) (segment-max subtraction is skipped: |score| <~ 60 <<
88 so exp cannot overflow, and alpha = es/sum(es) is exact), builds the
weighted one-hot W[e,j] = es_e * (dst_e == j) on DVE, and accumulates
    accT[h,j] += comp_c.T @ W_c      (TensorE, PSUM)
    den[j]    += W_c.T @ ones        (TensorE, PSUM)
then out_block = tanh((accT.T @ neigh_w) / den) and a contiguous DMA out.

dma_gather indices are int16, so ent_emb (50000 rows) src-gathers are split
into a lo (rows < 32768) and hi (rows >= 32768) gather per block, with edges
sorted by src inside each section (ascending HBM addresses).  Padded slots
repeat a real edge but carry dst_oh = 128 which matches no iota column, so
they contribute exactly zero to den/acc.
"""

import numpy as np

N_ENT = 50000
N_REL = 1000
N_EDGES = 600000
H = 128
P = 128
N_CORES = 8
NPC = N_ENT // N_CORES          # nodes per core
LO_ROWS = 32768                 # int16 gather split point

_cache = {}


def _build_program(npc, n_blocks, s_lo, s_hi, n_ent, n_rel, lo_rows):
    """Build the SPMD Bass/Tile program for one core shape."""
    import concourse.bacc as bacc
    import concourse.mybir as mybir
    import concourse.tile as tile

    f32 = mybir.dt.float32
    i16 = mybir.dt.int16
    S = s_lo + s_hi

    nc = bacc.Bacc("TRN2", target_bir_lowering=False, debug=False,
                   num_devices=N_CORES)

    ent = nc.dram_tensor("ent", [n_ent, H], f32, kind="ExternalInput")
    # this core's own node slice (dst rows) — per-core data, static local base
    ent_loc = nc.dram_tensor("ent_loc", [npc, H], f32, kind="ExternalInput")
    relt = nc.dram_tensor("relt", [n_rel, H], f32, kind="ExternalInput")
    w_in = nc.dram_tensor("w", [H, H], f32, kind="ExternalInput")
    iota_in = nc.dram_tensor("iota", [P, P], f32, kind="ExternalInput")
    sgi_in = nc.dram_tensor("src_gi", [P, n_blocks, S * 8], i16,
                            kind="ExternalInput")
    rgi_in = nc.dram_tensor("rel_gi", [P, n_blocks, S * 8], i16,
                            kind="ExternalInput")
    doh_in = nc.dram_tensor("dst_oh", [P, n_blocks, S], f32,
                            kind="ExternalInput")
    bnd_in = nc.dram_tensor("bounds", [P, n_blocks, 4], f32,
                            kind="ExternalInput")
    iote_in = nc.dram_tensor("iota_e", [P, S * P], f32,
                             kind="ExternalInput")
    out = nc.dram_tensor("out", [npc, H], f32, kind="ExternalOutput")

    import concourse.bass as bass

    def bc(ap, dims):
        # append/insert stride-0 dims: dims is the final [step,count] list
        return bass.AP(ap.tensor, ap.offset, dims)

    with tile.TileContext(nc) as tc:
        with (
            tc.tile_pool(name="const", bufs=1) as constp,
            tc.tile_pool(name="idx", bufs=1) as idxp,
            tc.tile_pool(name="data", bufs=2) as datap,
            tc.tile_pool(name="small", bufs=2) as smallp,
            tc.tile_pool(name="psum", bufs=2, space="PSUM") as psump,
            tc.tile_pool(name="psum1", bufs=2, space="PSUM") as psum1p,
            tc.tile_pool(name="psum2", bufs=2, space="PSUM") as psum2p,
        ):
            iota_t = constp.tile([P, P], f32)
            nc.sync.dma_start(iota_t[:], iota_in[:])
            w_t = constp.tile([H, H], f32)
            nc.sync.dma_start(w_t[:], w_in[:])
            ones_t = constp.tile([P, 1], f32)
            nc.vector.memset(ones_t[:], 1.0)

            sgi_t = idxp.tile([P, n_blocks, S * 8], i16)
            nc.sync.dma_start(sgi_t[:], sgi_in[:])
            rgi_t = idxp.tile([P, n_blocks, S * 8], i16)
            nc.sync.dma_start(rgi_t[:], rgi_in[:])
            doh_t = idxp.tile([P, n_blocks, S], f32)
            nc.sync.dma_start(doh_t[:], doh_in[:])
            bnd_t = idxp.tile([P, n_blocks, 4], f32)
            nc.sync.dma_start(bnd_t[:], bnd_in[:])
            iote_t = constp.tile([P, S * P], f32)
            nc.sync.dma_start(iote_t[:], iote_in[:])

            for b in range(n_blocks):
                base = b * P
                nodes_b = min(P, npc - base)

                src_rows = datap.tile([P, S, H], f32, tag="src")
                rel_rows = datap.tile([P, S, H], f32, tag="rel")
                dst_rows = datap.tile([P, S, H], f32, tag="dst")
                w_oh = datap.tile([P, S, H], f32, tag="W")

                if s_lo > 0:
                    nc.gpsimd.dma_gather(
                        src_rows[:, 0:s_lo, :], ent[0:lo_rows, :],
                        sgi_t[:, b, 0:s_lo * 8], s_lo * P, s_lo * P, H,
                        single_packet=False)
                if s_hi > 0:
                    nc.gpsimd.dma_gather(
                        src_rows[:, s_lo:S, :], ent[lo_rows:n_ent, :],
                        sgi_t[:, b, s_lo * 8:S * 8], s_hi * P, s_hi * P, H,
                        single_packet=False)
                nc.gpsimd.dma_gather(
                    rel_rows[:, :, :], relt[:, :],
                    rgi_t[:, b, :], S * P, S * P, H, single_packet=False)

                # dst rows: contiguous 128-row block load + one-hot PE expand.
                # OHT[j, e] = 1 iff slot e's dst == j; each node's slots are
                # two contiguous runs (lo/hi section), given by host bounds.
                eblk = datap.tile([P, H], f32, tag="eblk")
                if b < 2 or nodes_b < P:
                    nc.vector.memset(eblk[:], 0.0)
                nc.sync.dma_start(eblk[:nodes_b, :],
                                  ent_loc[base:base + nodes_b, :])
                oht = datap.tile([P, S * P], f32, tag="oht")
                t_a = datap.tile([P, S * P], f32, tag="t_a")
                t_b = datap.tile([P, S * P], f32, tag="t_b")

                def bnd(k):
                    ap = bnd_t[:, b, k:k + 1]
                    return bc(ap, [ap.ap[0], [0, S * P]])
                nc.vector.tensor_tensor(out=t_a[:], in0=iote_t[:],
                                        in1=bnd(0), op=mybir.AluOpType.is_ge)
                nc.vector.tensor_tensor(out=oht[:], in0=iote_t[:],
                                        in1=bnd(1), op=mybir.AluOpType.is_lt)
                nc.vector.tensor_tensor(out=t_a[:], in0=t_a[:], in1=oht[:],
                                        op=mybir.AluOpType.mult)
                nc.vector.tensor_tensor(out=t_b[:], in0=iote_t[:],
                                        in1=bnd(2), op=mybir.AluOpType.is_ge)
                nc.vector.tensor_tensor(out=oht[:], in0=iote_t[:],
                                        in1=bnd(3), op=mybir.AluOpType.is_lt)
                nc.vector.tensor_tensor(out=t_b[:], in0=t_b[:], in1=oht[:],
                                        op=mybir.AluOpType.mult)
                nc.vector.tensor_tensor(out=oht[:], in0=t_a[:], in1=t_b[:],
                                        op=mybir.AluOpType.add)
                for c in range(S):
                    dst_ps = psum2p.tile([P, H], f32, tag="dst_ps")
                    nc.tensor.matmul(
                        dst_ps[:], lhsT=oht[:, c * P:(c + 1) * P],
                        rhs=eblk[:], start=True, stop=True)
                    nc.scalar.copy(dst_rows[:, c, :], dst_ps[:])

                # comp = ent[src] * rel[rel_id]   (in-place over src_rows)
                nc.vector.tensor_tensor(
                    out=src_rows[:], in0=src_rows[:], in1=rel_rows[:],
                    op=mybir.AluOpType.mult)
                # prod = comp * ent[dst]          (in-place over dst_rows)
                nc.vector.tensor_tensor(
                    out=dst_rows[:], in0=src_rows[:], in1=dst_rows[:],
                    op=mybir.AluOpType.mult)
                score = smallp.tile([P, S], f32, tag="score")
                nc.vector.tensor_reduce(
                    out=score[:], in_=dst_rows[:],
                    axis=mybir.AxisListType.X, op=mybir.AluOpType.add)
                es = smallp.tile([P, S], f32, tag="es")
                nc.scalar.activation(
                    out=es[:], in_=score[:],
                    func=mybir.ActivationFunctionType.Exp)

                # one-hot: W[p, c, j] = (dst_oh[p, c] == j)
                doh_ap = doh_t[:, b, :]
                doh_b = bc(doh_ap, [doh_ap.ap[0], doh_ap.ap[1], [0, H]])
                iota_ap = iota_t[:]
                iota_b = bc(iota_ap, [iota_ap.ap[0], [0, S], iota_ap.ap[1]])
                nc.vector.tensor_tensor(
                    out=w_oh[:], in0=doh_b, in1=iota_b,
                    op=mybir.AluOpType.is_equal)
                # W *= es  (broadcast es over the one-hot columns)
                es_ap = es[:]
                es_b = bc(es_ap, [es_ap.ap[0], es_ap.ap[1], [0, H]])
                nc.vector.tensor_tensor(
                    out=w_oh[:], in0=w_oh[:], in1=es_b,
                    op=mybir.AluOpType.mult)

                # accT[h, j] = sum_c comp_c.T @ W_c
                acct_ps = psump.tile([P, P], f32, tag="accT")
                for c in range(S):
                    nc.tensor.matmul(
                        acct_ps[:], lhsT=src_rows[:, c, :], rhs=w_oh[:, c, :],
                        start=(c == 0), stop=(c == S - 1))
                # den[j] = sum_c W_c.T @ ones
                den_ps = psum1p.tile([P, 1], f32, tag="den")
                for c in range(S):
                    nc.tensor.matmul(
                        den_ps[:], lhsT=w_oh[:, c, :], rhs=ones_t[:],
                        start=(c == 0), stop=(c == S - 1))

                acct_sb = smallp.tile([P, P], f32, tag="acct_sb")
                nc.scalar.copy(acct_sb[:], acct_ps[:])
                den_sb = smallp.tile([P, 1], f32, tag="den_sb")
                nc.vector.tensor_scalar_max(den_sb[:], den_ps[:], 1e-30)
                rden = smallp.tile([P, 1], f32, tag="rden")
                nc.vector.reciprocal(rden[:], den_sb[:])

                out_ps = psump.tile([P, H], f32, tag="out_ps")
                nc.tensor.matmul(out_ps[:], lhsT=acct_sb[:], rhs=w_t[:],
                                 start=True, stop=True)
                out_sb = smallp.tile([P, H], f32, tag="out_sb")
                nc.scalar.activation(
                    out=out_sb[:], in_=out_ps[:],
                    func=mybir.ActivationFunctionType.Tanh, scale=rden[:])
                nc.sync.dma_start(out[base:base + nodes_b, :],
                                  out_sb[:nodes_b, :])

    nc.compile()
    return nc


def _idx_to_gather_layout(arr):
    """[S*128] int16 gather-position-ordered indices -> [128, S*8] tile."""
    a = arr.reshape(-1, 16).T.astype(np.int16)      # [16, S*8]
    return np.tile(a, (8, 1))                        # [128, S*8]


def _prep_inputs(ent_emb, rel_emb, neigh_w, src, dst, rel_id):
    """Partition edges by dst core/block, build per-core gather index arrays.

    Returns (in_maps, shape_key) where shape_key parameterizes the program.
    """
    src = np.asarray(src).astype(np.int64)
    dst = np.asarray(dst).astype(np.int64)
    rel_id = np.asarray(rel_id).astype(np.int64)
    n_blocks = (NPC + P - 1) // P

    order = np.argsort(dst, kind="stable")
    src_s, dst_s, rel_s = src[order], dst[order], rel_id[order]
    # per-(core,block) group id; monotone in dst since blocks nest in cores
    g_s = (dst_s // NPC) * n_blocks + (dst_s % NPC) // P
    n_gblocks = N_CORES * n_blocks
    bounds = np.searchsorted(g_s, np.arange(n_gblocks + 1))

    # first pass: per-block lo/hi counts -> global S_LO / S_HI
    max_lo = 1
    max_hi = 1
    lohi = []
    for g in range(n_gblocks):
        e0, e1 = bounds[g], bounds[g + 1]
        s_g = src_s[e0:e1]
        n_lo = int((s_g < LO_ROWS).sum())
        n_hi = int(e1 - e0 - n_lo)
        lohi.append((e0, e1, n_lo, n_hi))
        max_lo = max(max_lo, n_lo)
        max_hi = max(max_hi, n_hi)
    s_lo = (max_lo + P - 1) // P
    s_hi = (max_hi + P - 1) // P
    S = s_lo + s_hi

    in_maps = []
    for c in range(N_CORES):
        sgi = np.zeros((n_blocks, S * P), np.int16)
        rgi = np.zeros((n_blocks, S * P), np.int16)
        doh = np.full((n_blocks, S * P), float(P), np.float32)
        bnd = np.zeros((n_blocks, P, 4), np.float32)
        for b in range(n_blocks):
            g = c * n_blocks + b
            e0, e1, n_lo, n_hi = lohi[g]
            base = c * NPC + b * P
            s_g, d_g, r_g = src_s[e0:e1], dst_s[e0:e1], rel_s[e0:e1]
            is_lo = s_g < LO_ROWS
            for si, (sel, off, cap, sub) in enumerate(
                    ((is_lo, 0, s_lo * P, 0),
                     (~is_lo, s_lo * P, s_hi * P, LO_ROWS))):
                ss, dd, rr = s_g[sel], d_g[sel], r_g[sel]
                o2 = np.argsort(dd, kind="stable")  # dst-sorted: runs per node
                ss, dd, rr = ss[o2], dd[o2], rr[o2]
                n = len(ss)
                assert n <= cap
                sgi[b, off:off + n] = ss - sub
                rgi[b, off:off + n] = rr
                doh[b, off:off + n] = (dd - base).astype(np.float32)
                cum = np.searchsorted(dd - base, np.arange(P + 1)) + off
                bnd[b, :, 2 * si] = cum[:-1]
                bnd[b, :, 2 * si + 1] = cum[1:]
                if n < cap:  # pad with a repeat of a real edge (or zeros)
                    if n > 0:
                        sgi[b, off + n:off + cap] = ss[0] - sub
                        rgi[b, off + n:off + cap] = rr[0]
                    # doh stays 128 -> zero contribution
        # to device layouts
        sgi_l = np.stack([_idx_to_gather_layout(sgi[b]) for b in range(n_blocks)])
        rgi_l = np.stack([_idx_to_gather_layout(rgi[b]) for b in range(n_blocks)])
        doh_l = np.stack([doh[b].reshape(S, P).T for b in range(n_blocks)])
        iota = np.broadcast_to(np.arange(P, dtype=np.float32), (P, P)).copy()
        iota_e = np.broadcast_to(
            np.arange(S * P, dtype=np.float32), (P, S * P)).copy()
        in_maps.append({
            "ent": np.ascontiguousarray(ent_emb, np.float32),
            "ent_loc": np.ascontiguousarray(
                ent_emb[c * NPC:(c + 1) * NPC], np.float32),
            "relt": np.ascontiguousarray(rel_emb, np.float32),
            "w": np.ascontiguousarray(neigh_w, np.float32),
            "iota": iota,
            "iota_e": iota_e,
            "src_gi": np.ascontiguousarray(sgi_l.transpose(1, 0, 2)),
            "rel_gi": np.ascontiguousarray(rgi_l.transpose(1, 0, 2)),
            "bounds": np.ascontiguousarray(bnd.transpose(1, 0, 2)),
            "dst_oh": np.ascontiguousarray(
                doh_l.transpose(1, 0, 2).astype(np.float32)),
        })
    return in_maps, (NPC, n_blocks, s_lo, s_hi, N_ENT, N_REL, LO_ROWS)


LAST_RESULT = None


def _install_ntff_hook():
    """Provide the antenv.axon_hooks module the container's stub lacks, so
    run_bass_kernel_spmd(trace=True) can capture NTFF profiles via libaxon."""
    import sys
    import types
    if "antenv.axon_hooks" in sys.modules:
        return
    mod = types.ModuleType("antenv.axon_hooks")
    hook = [None]
    mod.set_axon_ntff_profile_hook = lambda h: hook.__setitem__(0, h)
    mod.get_axon_ntff_profile_hook = lambda: hook[0]
    sys.modules["antenv.axon_hooks"] = mod
    import antenv
    antenv.axon_hooks = mod
    try:
        from trn_agent_boot.trn_boot import _ntff_profile_via_ctypes
        h = _ntff_profile_via_ctypes("/opt/axon/libaxon_pjrt.so")
        if h is not None:
            mod.set_axon_ntff_profile_hook(lambda *a, **k: h(*a, **k))
    except Exception as e:  # degrade to no-trace
        print("ntff hook install failed:", e)


def kernel(ent_emb, rel_emb, neigh_w, src, dst, rel_id, _trace=False):
    global LAST_RESULT
    from concourse.bass_utils import run_bass_kernel_spmd
    if _trace:
        _install_ntff_hook()

    in_maps, key = _prep_inputs(ent_emb, rel_emb, neigh_w, src, dst, rel_id)
    if key not in _cache:
        _cache[key] = _build_program(*key)
    nc = _cache[key]
    res = run_bass_kernel_spmd(nc, in_maps, list(range(N_CORES)),
                               trace=_trace)
    LAST_RESULT = res
    return np.concatenate([r["out"] for r in res.results], axis=0)



# revision 11
# speedup vs baseline: 1.6628x; 1.2034x over previous
"""GNN message-passing (CompGCN-style edge-softmax) Trainium2 kernel.

Contract: kernel(**inputs) takes FULL unsharded inputs (ent_emb [50000,128] f32,
rel_emb [1000,128] f32, neigh_w [128,128] f32, src/dst/rel_id [600000] int) and
returns the FULL [50000,128] f32 output of:

    comp  = ent_emb[src] * rel_emb[rel_id]
    score = sum(comp * ent_emb[dst], -1)
    alpha = segment_softmax(score, dst)          # grouped by dst
    neigh = segment_sum(comp * alpha[:,None], dst)
    out   = tanh(neigh @ neigh_w)

Sharding: edges are sharded across the 8 cores BY DST RANGE (core c owns dst in
[c*6250,(c+1)*6250)), so segment max/sum are purely core-local and no
collective is needed; ent_emb/rel_emb/neigh_w are replicated.  Within a core,
edges are grouped by 128-node dst blocks; per block the kernel bulk-gathers
ent_emb[src] / rel_emb[rel] / ent_emb[dst] rows with dma_gather, computes
score -> es=exp(score) (segment-max subtraction is skipped: |score| <~ 60 <<
88 so exp cannot overflow, and alpha = es/sum(es) is exact), builds the
weighted one-hot W[e,j] = es_e * (dst_e == j) on DVE, and accumulates
    accT[h,j] += comp_c.T @ W_c      (TensorE, PSUM)
    den[j]    += W_c.T @ ones        (TensorE, PSUM)
then out_block = tanh((accT.T @ neigh_w) / den) and a contiguous DMA out.

dma_gather indices are int16, so ent_emb (50000 rows) src-gathers are split
into a lo (rows < 32768) and hi (rows >= 32768) gather per block, with edges
sorted by src inside each section (ascending HBM addresses).  Padded slots
repeat a real edge but carry dst_oh = 128 which matches no iota column, so
they contribute exactly zero to den/acc.
"""

import numpy as np

N_ENT = 50000
N_REL = 1000
N_EDGES = 600000
H = 128
P = 128
N_CORES = 8
NPC = N_ENT // N_CORES          # nodes per core
LO_ROWS = 32768                 # int16 gather split point

_cache = {}


def _build_program(npc, n_blocks, s_lo, s_hi, n_ent, n_rel, lo_rows):
    """Build the SPMD Bass/Tile program for one core shape."""
    import concourse.bacc as bacc
    import concourse.mybir as mybir
    import concourse.tile as tile

    f32 = mybir.dt.float32
    i16 = mybir.dt.int16
    S = s_lo + s_hi

    nc = bacc.Bacc("TRN2", target_bir_lowering=False, debug=False,
                   num_devices=N_CORES)

    ent = nc.dram_tensor("ent", [n_ent, H], f32, kind="ExternalInput")
    # this core's own node slice (dst rows) — per-core data, static local base
    ent_loc = nc.dram_tensor("ent_loc", [npc, H], f32, kind="ExternalInput")
    relt = nc.dram_tensor("relt", [n_rel, H], f32, kind="ExternalInput")
    w_in = nc.dram_tensor("w", [H, H], f32, kind="ExternalInput")
    iota_in = nc.dram_tensor("iota", [P, P], f32, kind="ExternalInput")
    sgi_in = nc.dram_tensor("src_gi", [P, n_blocks, S * 8], i16,
                            kind="ExternalInput")
    rgi_in = nc.dram_tensor("rel_gi", [P, n_blocks, S * 8], i16,
                            kind="ExternalInput")
    doh_in = nc.dram_tensor("dst_oh", [P, n_blocks, S], f32,
                            kind="ExternalInput")
    bnd_in = nc.dram_tensor("bounds", [P, n_blocks, 4], f32,
                            kind="ExternalInput")
    iote_in = nc.dram_tensor("iota_e", [P, S * P], f32,
                             kind="ExternalInput")
    out = nc.dram_tensor("out", [npc, H], f32, kind="ExternalOutput")

    import concourse.bass as bass

    def bc(ap, dims):
        # append/insert stride-0 dims: dims is the final [step,count] list
        return bass.AP(ap.tensor, ap.offset, dims)

    with tile.TileContext(nc) as tc:
        with (
            tc.tile_pool(name="const", bufs=1) as constp,
            tc.tile_pool(name="idx", bufs=1) as idxp,
            tc.tile_pool(name="gath", bufs=4) as gathp,
            tc.tile_pool(name="data", bufs=2) as datap,
            tc.tile_pool(name="small", bufs=2) as smallp,
            tc.tile_pool(name="psum", bufs=2, space="PSUM") as psump,
            tc.tile_pool(name="psum1", bufs=2, space="PSUM") as psum1p,
            tc.tile_pool(name="psum2", bufs=2, space="PSUM") as psum2p,
        ):
            iota_t = constp.tile([P, P], f32)
            nc.sync.dma_start(iota_t[:], iota_in[:])
            w_t = constp.tile([H, H], f32)
            nc.sync.dma_start(w_t[:], w_in[:])
            ones_t = constp.tile([P, 1], f32)
            nc.vector.memset(ones_t[:], 1.0)

            sgi_t = idxp.tile([P, n_blocks, S * 8], i16)
            nc.sync.dma_start(sgi_t[:], sgi_in[:])
            rgi_t = idxp.tile([P, n_blocks, S * 8], i16)
            nc.sync.dma_start(rgi_t[:], rgi_in[:])
            doh_t = idxp.tile([P, n_blocks, S], f32)
            nc.sync.dma_start(doh_t[:], doh_in[:])
            bnd_t = idxp.tile([P, n_blocks, 4], f32)
            nc.sync.dma_start(bnd_t[:], bnd_in[:])
            iote_t = constp.tile([P, S * P], f32)
            nc.sync.dma_start(iote_t[:], iote_in[:])

            for b in range(n_blocks):
                base = b * P
                nodes_b = min(P, npc - base)

                src_rows = gathp.tile([P, S, H], f32, tag="src")
                rel_rows = gathp.tile([P, S, H], f32, tag="rel")
                dst_rows = datap.tile([P, S, H], f32, tag="dst")
                w_oh = datap.tile([P, S, H], f32, tag="W")

                if s_lo > 0:
                    nc.gpsimd.dma_gather(
                        src_rows[:, 0:s_lo, :], ent[0:lo_rows, :],
                        sgi_t[:, b, 0:s_lo * 8], s_lo * P, s_lo * P, H,
                        single_packet=False)
                if s_hi > 0:
                    nc.gpsimd.dma_gather(
                        src_rows[:, s_lo:S, :], ent[lo_rows:n_ent, :],
                        sgi_t[:, b, s_lo * 8:S * 8], s_hi * P, s_hi * P, H,
                        single_packet=False)
                nc.gpsimd.dma_gather(
                    rel_rows[:, :, :], relt[:, :],
                    rgi_t[:, b, :], S * P, S * P, H, single_packet=False)

                # dst rows: contiguous 128-row block load + one-hot PE expand.
                # OHT[j, e] = 1 iff slot e's dst == j; each node's slots are
                # two contiguous runs (lo/hi section), given by host bounds.
                eblk = datap.tile([P, H], f32, tag="eblk")
                if b < 2 or nodes_b < P:
                    nc.vector.memset(eblk[:], 0.0)
                nc.sync.dma_start(eblk[:nodes_b, :],
                                  ent_loc[base:base + nodes_b, :])
                oht = datap.tile([P, S * P], f32, tag="oht")
                t_a = datap.tile([P, S * P], f32, tag="t_a")
                t_b = datap.tile([P, S * P], f32, tag="t_b")

                def bnd(k):
                    ap = bnd_t[:, b, k:k + 1]
                    return bc(ap, [ap.ap[0], [0, S * P]])
                nc.vector.tensor_tensor(out=t_a[:], in0=iote_t[:],
                                        in1=bnd(0), op=mybir.AluOpType.is_ge)
                nc.vector.tensor_tensor(out=oht[:], in0=iote_t[:],
                                        in1=bnd(1), op=mybir.AluOpType.is_lt)
                nc.vector.tensor_tensor(out=t_a[:], in0=t_a[:], in1=oht[:],
                                        op=mybir.AluOpType.mult)
                nc.vector.tensor_tensor(out=t_b[:], in0=iote_t[:],
                                        in1=bnd(2), op=mybir.AluOpType.is_ge)
                nc.vector.tensor_tensor(out=oht[:], in0=iote_t[:],
                                        in1=bnd(3), op=mybir.AluOpType.is_lt)
                nc.vector.tensor_tensor(out=t_b[:], in0=t_b[:], in1=oht[:],
                                        op=mybir.AluOpType.mult)
                nc.vector.tensor_tensor(out=oht[:], in0=t_a[:], in1=t_b[:],
                                        op=mybir.AluOpType.add)
                for c in range(S):
                    dst_ps = psum2p.tile([P, H], f32, tag="dst_ps")
                    nc.tensor.matmul(
                        dst_ps[:], lhsT=oht[:, c * P:(c + 1) * P],
                        rhs=eblk[:], start=True, stop=True)
                    nc.scalar.copy(dst_rows[:, c, :], dst_ps[:])

                # comp = ent[src] * rel[rel_id]   (in-place over src_rows)
                nc.vector.tensor_tensor(
                    out=src_rows[:], in0=src_rows[:], in1=rel_rows[:],
                    op=mybir.AluOpType.mult)
                # prod = comp * ent[dst]          (in-place over dst_rows)
                nc.vector.tensor_tensor(
                    out=dst_rows[:], in0=src_rows[:], in1=dst_rows[:],
                    op=mybir.AluOpType.mult)
                score = smallp.tile([P, S], f32, tag="score")
                nc.vector.tensor_reduce(
                    out=score[:], in_=dst_rows[:],
                    axis=mybir.AxisListType.X, op=mybir.AluOpType.add)
                es = smallp.tile([P, S], f32, tag="es")
                nc.scalar.activation(
                    out=es[:], in_=score[:],
                    func=mybir.ActivationFunctionType.Exp)

                # one-hot: W[p, c, j] = (dst_oh[p, c] == j)
                doh_ap = doh_t[:, b, :]
                doh_b = bc(doh_ap, [doh_ap.ap[0], doh_ap.ap[1], [0, H]])
                iota_ap = iota_t[:]
                iota_b = bc(iota_ap, [iota_ap.ap[0], [0, S], iota_ap.ap[1]])
                nc.vector.tensor_tensor(
                    out=w_oh[:], in0=doh_b, in1=iota_b,
                    op=mybir.AluOpType.is_equal)
                # W *= es  (broadcast es over the one-hot columns)
                es_ap = es[:]
                es_b = bc(es_ap, [es_ap.ap[0], es_ap.ap[1], [0, H]])
                nc.vector.tensor_tensor(
                    out=w_oh[:], in0=w_oh[:], in1=es_b,
                    op=mybir.AluOpType.mult)

                # accT[h, j] = sum_c comp_c.T @ W_c
                acct_ps = psump.tile([P, P], f32, tag="accT")
                for c in range(S):
                    nc.tensor.matmul(
                        acct_ps[:], lhsT=src_rows[:, c, :], rhs=w_oh[:, c, :],
                        start=(c == 0), stop=(c == S - 1))
                # den[j] = sum_c W_c.T @ ones
                den_ps = psum1p.tile([P, 1], f32, tag="den")
                for c in range(S):
                    nc.tensor.matmul(
                        den_ps[:], lhsT=w_oh[:, c, :], rhs=ones_t[:],
                        start=(c == 0), stop=(c == S - 1))

                acct_sb = smallp.tile([P, P], f32, tag="acct_sb")
                nc.scalar.copy(acct_sb[:], acct_ps[:])
                den_sb = smallp.tile([P, 1], f32, tag="den_sb")
                nc.vector.tensor_scalar_max(den_sb[:], den_ps[:], 1e-30)
                rden = smallp.tile([P, 1], f32, tag="rden")
                nc.vector.reciprocal(rden[:], den_sb[:])

                out_ps = psump.tile([P, H], f32, tag="out_ps")
                nc.tensor.matmul(out_ps[:], lhsT=acct_sb[:], rhs=w_t[:],
                                 start=True, stop=True)
                out_sb = smallp.tile([P, H], f32, tag="out_sb")
                nc.scalar.activation(
                    out=out_sb[:], in_=out_ps[:],
                    func=mybir.ActivationFunctionType.Tanh, scale=rden[:])
                nc.sync.dma_start(out[base:base + nodes_b, :],
                                  out_sb[:nodes_b, :])

    nc.compile()
    return nc


def _idx_to_gather_layout(arr):
    """[S*128] int16 gather-position-ordered indices -> [128, S*8] tile."""
    a = arr.reshape(-1, 16).T.astype(np.int16)      # [16, S*8]
    return np.tile(a, (8, 1))                        # [128, S*8]


def _prep_inputs(ent_emb, rel_emb, neigh_w, src, dst, rel_id):
    """Partition edges by dst core/block, build per-core gather index arrays.

    Returns (in_maps, shape_key) where shape_key parameterizes the program.
    """
    src = np.asarray(src).astype(np.int64)
    dst = np.asarray(dst).astype(np.int64)
    rel_id = np.asarray(rel_id).astype(np.int64)
    n_blocks = (NPC + P - 1) // P

    order = np.argsort(dst, kind="stable")
    src_s, dst_s, rel_s = src[order], dst[order], rel_id[order]
    # per-(core,block) group id; monotone in dst since blocks nest in cores
    g_s = (dst_s // NPC) * n_blocks + (dst_s % NPC) // P
    n_gblocks = N_CORES * n_blocks
    bounds = np.searchsorted(g_s, np.arange(n_gblocks + 1))

    # first pass: per-block lo/hi counts -> global S_LO / S_HI
    max_lo = 1
    max_hi = 1
    lohi = []
    for g in range(n_gblocks):
        e0, e1 = bounds[g], bounds[g + 1]
        s_g = src_s[e0:e1]
        n_lo = int((s_g < LO_ROWS).sum())
        n_hi = int(e1 - e0 - n_lo)
        lohi.append((e0, e1, n_lo, n_hi))
        max_lo = max(max_lo, n_lo)
        max_hi = max(max_hi, n_hi)
    s_lo = (max_lo + P - 1) // P
    s_hi = (max_hi + P - 1) // P
    S = s_lo + s_hi

    in_maps = []
    for c in range(N_CORES):
        sgi = np.zeros((n_blocks, S * P), np.int16)
        rgi = np.zeros((n_blocks, S * P), np.int16)
        doh = np.full((n_blocks, S * P), float(P), np.float32)
        bnd = np.zeros((n_blocks, P, 4), np.float32)
        for b in range(n_blocks):
            g = c * n_blocks + b
            e0, e1, n_lo, n_hi = lohi[g]
            base = c * NPC + b * P
            s_g, d_g, r_g = src_s[e0:e1], dst_s[e0:e1], rel_s[e0:e1]
            is_lo = s_g < LO_ROWS
            for si, (sel, off, cap, sub) in enumerate(
                    ((is_lo, 0, s_lo * P, 0),
                     (~is_lo, s_lo * P, s_hi * P, LO_ROWS))):
                ss, dd, rr = s_g[sel], d_g[sel], r_g[sel]
                o2 = np.argsort(dd, kind="stable")  # dst-sorted: runs per node
                ss, dd, rr = ss[o2], dd[o2], rr[o2]
                n = len(ss)
                assert n <= cap
                sgi[b, off:off + n] = ss - sub
                rgi[b, off:off + n] = rr
                doh[b, off:off + n] = (dd - base).astype(np.float32)
                cum = np.searchsorted(dd - base, np.arange(P + 1)) + off
                bnd[b, :, 2 * si] = cum[:-1]
                bnd[b, :, 2 * si + 1] = cum[1:]
                if n < cap:  # pad with a repeat of a real edge (or zeros)
                    if n > 0:
                        sgi[b, off + n:off + cap] = ss[0] - sub
                        rgi[b, off + n:off + cap] = rr[0]
                    # doh stays 128 -> zero contribution
        # to device layouts
        sgi_l = np.stack([_idx_to_gather_layout(sgi[b]) for b in range(n_blocks)])
        rgi_l = np.stack([_idx_to_gather_layout(rgi[b]) for b in range(n_blocks)])
        doh_l = np.stack([doh[b].reshape(S, P).T for b in range(n_blocks)])
        iota = np.broadcast_to(np.arange(P, dtype=np.float32), (P, P)).copy()
        iota_e = np.broadcast_to(
            np.arange(S * P, dtype=np.float32), (P, S * P)).copy()
        in_maps.append({
            "ent": np.ascontiguousarray(ent_emb, np.float32),
            "ent_loc": np.ascontiguousarray(
                ent_emb[c * NPC:(c + 1) * NPC], np.float32),
            "relt": np.ascontiguousarray(rel_emb, np.float32),
            "w": np.ascontiguousarray(neigh_w, np.float32),
            "iota": iota,
            "iota_e": iota_e,
            "src_gi": np.ascontiguousarray(sgi_l.transpose(1, 0, 2)),
            "rel_gi": np.ascontiguousarray(rgi_l.transpose(1, 0, 2)),
            "bounds": np.ascontiguousarray(bnd.transpose(1, 0, 2)),
            "dst_oh": np.ascontiguousarray(
                doh_l.transpose(1, 0, 2).astype(np.float32)),
        })
    return in_maps, (NPC, n_blocks, s_lo, s_hi, N_ENT, N_REL, LO_ROWS)


LAST_RESULT = None


def _install_ntff_hook():
    """Provide the antenv.axon_hooks module the container's stub lacks, so
    run_bass_kernel_spmd(trace=True) can capture NTFF profiles via libaxon."""
    import sys
    import types
    if "antenv.axon_hooks" in sys.modules:
        return
    mod = types.ModuleType("antenv.axon_hooks")
    hook = [None]
    mod.set_axon_ntff_profile_hook = lambda h: hook.__setitem__(0, h)
    mod.get_axon_ntff_profile_hook = lambda: hook[0]
    sys.modules["antenv.axon_hooks"] = mod
    import antenv
    antenv.axon_hooks = mod
    try:
        from trn_agent_boot.trn_boot import _ntff_profile_via_ctypes
        h = _ntff_profile_via_ctypes("/opt/axon/libaxon_pjrt.so")
        if h is not None:
            mod.set_axon_ntff_profile_hook(lambda *a, **k: h(*a, **k))
    except Exception as e:  # degrade to no-trace
        print("ntff hook install failed:", e)


def kernel(ent_emb, rel_emb, neigh_w, src, dst, rel_id, _trace=False):
    global LAST_RESULT
    from concourse.bass_utils import run_bass_kernel_spmd
    if _trace:
        _install_ntff_hook()

    in_maps, key = _prep_inputs(ent_emb, rel_emb, neigh_w, src, dst, rel_id)
    if key not in _cache:
        _cache[key] = _build_program(*key)
    nc = _cache[key]
    res = run_bass_kernel_spmd(nc, in_maps, list(range(N_CORES)),
                               trace=_trace)
    LAST_RESULT = res
    return np.concatenate([r["out"] for r in res.results], axis=0)

